# revision 1
# baseline (speedup 1.0000x reference)
"""Multi-head causal attention (B=4, T=2048, D=1024, H=16) on 8 Trainium2
NeuronCores.

Sharding: core c -> (batch = c//2, head-group = c%2, 8 heads each).
Each core: QKV projection for its batch/head-group, causal attention,
partial output projection over its heads' rows of w_proj, then a 2-way
ReduceScatter with its pair core (same batch, other head-group) that
splits output channels. Host reassembles with a transpose+concat only.

On-chip orientation is "transposed" throughout (channels on partitions,
tokens on the free dim):
  xT   (D, T)    bf16, PE transpose-mode from natural x tiles
  qkT  (1024, T) = wqk.T @ xT  (q rows pre-scaled by 1/sqrt(Dh) on host)
  v    (T, 512)  natural, with an extra all-ones column per head slot
  sT   (k, q)    = K_tile @ qT  -> exp on ScalarE -> es (bf16)
  outT (65, q)   = [v|1].T @ es  (fp32 PSUM; row 64 = softmax denom)
  finalT (1024, q) = wp.T @ (outT / denom) + bias, -> ReduceScatter

The QKV projection is produced in 512-token chunks and chunk qc+1 is
woven between the attention head-pairs of q-macro qc, so the PE stream
stays dense (exp latency on ScalarE is hidden by projection matmuls)
and the PE clock stays un-throttled. AV matmuls run one pipeline step
behind scores/exp. Causal mask: strictly-upper triangular 128x128
multiply on boundary tiles only; future k-tiles are never computed.
Matmul operands are bf16 (1 PE cycle/row, fp32 accumulate); softmax
statistics stay fp32; partials/collectives are bf16.
"""

import numpy as np
import ml_dtypes

import concourse.bass as bass
from concourse import bacc
import concourse.mybir as mybir
import concourse.tile as tile
from concourse.bass_utils import run_bass_kernel_spmd
from concourse.masks import make_identity, make_upper_triangular

B, T, D = 4, 2048, 1024
H_TOT, DH = 16, 64
HL = 8          # heads per core
P = 128
ND = D // P     # 8 d-tiles
NT = T // P     # 16 token tiles
NQ = T // 512   # 4 q-macros
F32 = mybir.dt.float32
BF16 = mybir.dt.bfloat16
AF = mybir.ActivationFunctionType
NP_BF16 = ml_dtypes.bfloat16

REPLICA_GROUPS = [[0, 1], [2, 3], [4, 5], [6, 7]]


def build_bass():
    nc = bacc.Bacc(None, target_bir_lowering=False, num_devices=8)

    x = nc.dram_tensor("x", [T, D], BF16, kind="ExternalInput")
    wqk = nc.dram_tensor("wqk", [D, 1024], BF16, kind="ExternalInput")
    wv = nc.dram_tensor("wv", [D, 512], BF16, kind="ExternalInput")
    wp = nc.dram_tensor("wp", [512, D], BF16, kind="ExternalInput")
    bias = nc.dram_tensor("bias", [D], F32, kind="ExternalInput")
    out = nc.dram_tensor("out", [512, T], BF16, kind="ExternalOutput")

    with tile.TileContext(nc, num_cores=8) as tc:
        with (
            tc.tile_pool(name="const", bufs=1) as const_pool,
            tc.tile_pool(name="dram", bufs=1, space="DRAM") as dram_pool,
            tc.tile_pool(name="persist", bufs=1) as persist,
            tc.tile_pool(name="wp_pool", bufs=1) as wp_pool,
            tc.tile_pool(name="es_pool", bufs=10) as es_pool,
            tc.tile_pool(name="oh_pool", bufs=2) as oh_pool,
            tc.tile_pool(name="ohu_pool", bufs=3) as ohu_pool,
            tc.tile_pool(name="cs_pool", bufs=2) as cs_pool,
            tc.tile_pool(name="rb_pool", bufs=3) as rb_pool,
            tc.tile_pool(name="po_pool", bufs=3) as po_pool,
            tc.tile_pool(name="ps_s", bufs=2, space="PSUM") as ps_s,
            tc.tile_pool(name="ps_av", bufs=2, space="PSUM") as ps_av,
            tc.tile_pool(name="ps_mm", bufs=2, space="PSUM") as ps_mm,
        ):
            tri = const_pool.tile([P, P], BF16)
            make_upper_triangular(nc, tri, val=1.0, diag=True)
            ident = const_pool.tile([P, P], BF16)
            make_identity(nc, ident)
            bias_sb = const_pool.tile([P, ND], F32)
            nc.gpsimd.dma_start(out=bias_sb,
                                in_=bias.ap().rearrange("(n p) -> p n", p=P))

            qkT = persist.tile([P, ND, T], BF16, name="qkT")
            v1 = persist.tile([P, NT, HL, DH + 1], BF16, name="v1")
            ones_sb = const_pool.tile([P, NT, HL, 1], F32)
            nc.vector.memset(ones_sb, 1.0)
            nc.vector.tensor_copy(out=v1[:, :, :, DH:DH + 1], in_=ones_sb)

            wp_sb = wp_pool.tile([P, 4, D], BF16)
            nc.gpsimd.dma_start(
                out=wp_sb, in_=wp.ap().rearrange("(n p) m -> p n m", p=P))

            def attention(qm, weave):
                """Attention for q-macro qm; pulls from `weave` (an iterator
                of thunks emitting projection matmul groups) between pairs."""
                nkt = 4 * qm + 4
                nb = nkt - 4
                oh_sb = oh_pool.tile([P, 4, 512], BF16, name="oh_sb")

                for hp in range(4):
                    heads = (2 * hp, 2 * hp + 1)
                    out_ps = {}
                    for h in heads:
                        out_ps[h] = ps_av.tile([DH + 1, 512], F32,
                                               name=f"out_ps{h % 2}",
                                               tag="ps_av")

                    def av_mm(h, kt, src):
                        nc.tensor.matmul(
                            out_ps[h][:, max(0, P * kt - 512 * qm):],
                            lhsT=v1[:, kt, h, :],
                            rhs=src,
                            start=(kt == 0), stop=(kt == nkt - 1),
                            skip_group_check=True)

                    def scores_mm(h, kt, dst, qoff):
                        hi = (h % 2) * DH
                        nc.tensor.matmul(
                            dst,
                            lhsT=qkT[hi:hi + DH, 4 + h // 2,
                                     kt * P:(kt + 1) * P],
                            rhs=qkT[hi:hi + DH, h // 2,
                                    qm * 512 + qoff:(qm + 1) * 512],
                            start=True, stop=True)

                    # software pipeline: AVs one step behind scores/exp
                    pend = []

                    def flush():
                        for h_, kt_, src_ in pend:
                            av_mm(h_, kt_, src_)
                        pend.clear()

                    for kt2 in range(nb // 2):   # non-boundary, 2 per exp
                        kts = (2 * kt2, 2 * kt2 + 1)
                        step = []
                        for h in heads:
                            s2 = ps_s.tile([P, 2, 512], F32, name="s2",
                                           tag="ps_s")
                            for j, kt in enumerate(kts):
                                scores_mm(h, kt, s2[:, j, :], 0)
                            e2 = es_pool.tile([P, 2, 512], BF16, name="es",
                                              tag="es")
                            nc.scalar.activation(out=e2, in_=s2, func=AF.Exp)
                            for j, kt in enumerate(kts):
                                step.append((h, kt, e2[:, j, :]))
                        flush()
                        pend.extend(step)
                    for kt in range(nb, nkt):    # boundary, masked
                        qoff = P * kt - 512 * qm
                        step = []
                        for h in heads:
                            sb_ps = ps_s.tile([P, 2, 512], F32, name="sb",
                                              tag="ps_s")
                            scores_mm(h, kt, sb_ps[:, 0, qoff:], qoff)
                            e2 = es_pool.tile([P, 2, 512], BF16, name="esb",
                                              tag="es")
                            nc.scalar.activation(
                                out=e2[:, 0, qoff:], in_=sb_ps[:, 0, qoff:],
                                func=AF.Exp)
                            nc.vector.tensor_mul(
                                e2[:, 0, qoff:qoff + P],
                                e2[:, 0, qoff:qoff + P], tri)
                            step.append((h, kt, e2[:, 0, qoff:]))
                        flush()
                        pend.extend(step)
                    flush()

                    # evacuate PSUM accumulators, then normalize
                    ohu = ohu_pool.tile([P, 512], F32, name="ohu")
                    cs = cs_pool.tile([P, 2, 512], F32, name="cs")
                    for i, h in enumerate(heads):
                        hi = (h % 2) * DH
                        nc.vector.tensor_copy(
                            out=ohu[hi:hi + DH, :], in_=out_ps[h][0:DH, :])
                        nc.vector.tensor_copy(
                            out=cs[0:1, i, :], in_=out_ps[h][DH:DH + 1, :])
                    csw = cs_pool.tile([P, 8], F32, name="csw", tag="csw")
                    nc.sync.dma_start(out=csw, in_=cs[0:1, :, :])
                    nc.vector.reciprocal(out=csw, in_=csw)
                    rcs = cs_pool.tile([P, 2, 512], F32, name="rcs", tag="rcs")
                    nc.sync.dma_start(out=rcs[0:1, :, :], in_=csw)
                    rb = rb_pool.tile([P, 2, 512], F32, name="rb")
                    nc.gpsimd.partition_broadcast(rb[:, 0, :], rcs[0:1, 0, :])
                    nc.gpsimd.partition_broadcast(rb[:, 1, :], rcs[0:1, 1, :])
                    for i, h in enumerate(heads):
                        hi = (h % 2) * DH
                        nc.vector.tensor_mul(
                            oh_sb[hi:hi + DH, hp, :],
                            ohu[hi:hi + DH, :], rb[hi:hi + DH, i, :])

                    # weave in dense projection work for the next chunk
                    if weave is not None:
                        for _ in range(3):
                            thunk = next(weave, None)
                            if thunk is None:
                                break
                            thunk()
                return oh_sb

            def oc_group(qm, oh_sb, partial, oc, j):
                ps = ps_mm.tile([P, 512], F32, name="ps_f", tag="ps_mm")
                for dt in range(4):
                    nc.tensor.matmul(
                        ps,
                        lhsT=wp_sb[:, dt, oc * P:(oc + 1) * P],
                        rhs=oh_sb[:, dt, :],
                        start=(dt == 0), stop=(dt == 3))
                po = po_pool.tile([P, 512], BF16, name="po")
                nc.vector.tensor_scalar_add(
                    out=po, in0=ps, scalar1=bias_sb[:, oc:oc + 1])
                nc.sync.dma_start(out=partial[j * P:(j + 1) * P, :], in_=po)

            def rs_chunk(qm, chunk, partial):
                rs = dram_pool.tile([256, 512], BF16, name=f"rs{qm}_{chunk}",
                                    tag=f"rs{qm}_{chunk}")
                nc.gpsimd.collective_compute(
                    "ReduceScatter", mybir.AluOpType.add,
                    replica_groups=REPLICA_GROUPS,
                    ins=[partial[:, :]], outs=[rs[:, :]])
                nc.sync.dma_start(
                    out=out.ap()[chunk * 256:(chunk + 1) * 256,
                                 qm * 512:(qm + 1) * 512], in_=rs)

            def out_proj_groups(qm, oh_sb):
                """Generator of thunks: 2 halves x (4 oc groups + RS)."""
                for chunk in range(2):
                    partial = dram_pool.tile(
                        [512, 512], BF16, name=f"partial{qm}_{chunk}",
                        tag=f"partial{qm}_{chunk}")
                    for j in range(4):
                        oc = 4 * chunk + j
                        yield (lambda qm=qm, oh=oh_sb, pa=partial, oc=oc, j=j:
                               oc_group(qm, oh, pa, oc, j))
                    yield (lambda qm=qm, ch=chunk, pa=partial:
                           rs_chunk(qm, ch, pa))

            def out_proj(qm, oh_sb):
                for thunk in out_proj_groups(qm, oh_sb):
                    thunk()

            # ---- projection machinery (chunked by 512 tokens) ----
            with (
                tc.tile_pool(name="xT_pool", bufs=1) as xT_pool,
                tc.tile_pool(name="wa_pool", bufs=1) as wa_pool,
                tc.tile_pool(name="xin", bufs=6) as xin_pool,
            ):
                xT = xT_pool.tile([P, ND, T], BF16)
                wv_sb = wa_pool.tile([P, ND, 512], BF16)
                nc.gpsimd.dma_start(
                    out=wv_sb, in_=wv.ap().rearrange("(n p) m -> p n m", p=P))
                wqk_sb = wa_pool.tile([P, ND, 1024], BF16)
                nc.gpsimd.dma_start(
                    out=wqk_sb, in_=wqk.ap().rearrange("(n p) m -> p n m", p=P))

                def transpose_group(tg):
                    xins = []
                    for j in range(4):
                        tt = tg * 4 + j
                        xin = xin_pool.tile([P, D], BF16, name=f"xin{tt}",
                                            tag="xin")
                        nc.sync.dma_start(
                            out=xin, in_=x.ap()[tt * P:(tt + 1) * P, :])
                        xins.append(xin)
                    for dd in range(ND):
                        ps = ps_mm.tile([P, 512], BF16, name="ps_tr",
                                        tag="ps_mm")
                        for j in range(4):
                            nc.tensor.transpose(
                                ps[:, j * P:(j + 1) * P],
                                xins[j][:, dd * P:(dd + 1) * P], ident)
                        nc.vector.tensor_copy(
                            out=xT[:, dd, tg * 512:(tg + 1) * 512], in_=ps)

                def qk_group(pt, qc):
                    ps = ps_mm.tile([P, 512], F32, name="ps_qk", tag="ps_mm")
                    for dd in range(ND):
                        nc.tensor.matmul(
                            ps,
                            lhsT=wqk_sb[:, dd, pt * P:(pt + 1) * P],
                            rhs=xT[:, dd, qc * 512:(qc + 1) * 512],
                            start=(dd == 0), stop=(dd == ND - 1))
                    nc.vector.tensor_copy(
                        out=qkT[:, pt, qc * 512:(qc + 1) * 512], in_=ps)

                def v_group(tt):
                    ps = ps_mm.tile([P, 512], F32, name="ps_v", tag="ps_mm")
                    for dd in range(ND):
                        nc.tensor.matmul(
                            ps,
                            lhsT=xT[:, dd, tt * P:(tt + 1) * P],
                            rhs=wv_sb[:, dd, :],
                            start=(dd == 0), stop=(dd == ND - 1))
                    nc.vector.tensor_copy(
                        out=v1[:, tt, :, 0:DH],
                        in_=ps.rearrange("p (h d) -> p h d", h=HL))

                def proj_chunk_groups(qc):
                    for pt in range(8):
                        yield lambda pt=pt: qk_group(pt, qc)
                    for tt in range(4 * qc, 4 * qc + 4):
                        yield lambda tt=tt: v_group(tt)

                # prelude: transpose all of x, project chunk 0
                for tg in range(4):
                    transpose_group(tg)
                for g in proj_chunk_groups(0):
                    g()

                # q-macros 0..2, weaving in the next chunk's projections
                oh2 = None
                for qm in range(3):
                    weave = proj_chunk_groups(qm + 1)
                    oh_sb = attention(qm, weave)
                    for thunk in weave:  # drain leftovers
                        thunk()
                    if qm < 2:
                        out_proj(qm, oh_sb)
                    else:
                        oh2 = oh_sb

            # q-macro 3: weave qm2's out-projection between its pairs
            weave3 = out_proj_groups(2, oh2)
            oh_sb = attention(3, weave3)
            for thunk in weave3:
                thunk()
            out_proj(3, oh_sb)

    nc.finalize()
    return nc


_NC_CACHE = None


def _get_nc():
    global _NC_CACHE
    if _NC_CACHE is None:
        _NC_CACHE = build_bass()
    return _NC_CACHE


def _make_in_maps(x, w_qkv, w_proj, b_proj):
    x = np.asarray(x, np.float32)
    w_qkv = np.asarray(w_qkv, np.float32)
    w_proj = np.asarray(w_proj, np.float32)
    b_proj = np.asarray(b_proj, np.float32)
    wq, wk, wv_full = w_qkv[:, :D], w_qkv[:, D:2 * D], w_qkv[:, 2 * D:]
    scale = DH ** -0.5
    in_maps = []
    for c in range(8):
        b, g = c // 2, c % 2
        cols = slice(g * 512, (g + 1) * 512)
        wqk_c = np.concatenate([wq[:, cols] * scale, wk[:, cols]], axis=1)
        in_maps.append({
            "x": np.ascontiguousarray(x[b]).astype(NP_BF16),
            "wqk": np.ascontiguousarray(wqk_c).astype(NP_BF16),
            "wv": np.ascontiguousarray(wv_full[:, cols]).astype(NP_BF16),
            "wp": np.ascontiguousarray(
                w_proj[g * 512:(g + 1) * 512, :]).astype(NP_BF16),
            "bias": b_proj if g == 0 else np.zeros_like(b_proj),
        })
    return in_maps


def _assemble(results):
    out = np.empty((B, T, D), np.float32)
    for c in range(8):
        b, r = c // 2, c % 2
        res = results[c]["out"].astype(np.float32)
        # half h rows [256h, 256h+256) = global outcols
        # [512h + 256r, 512h + 256r + 256)
        for ch in range(2):
            out[b, :, 512 * ch + 256 * r:512 * ch + 256 * r + 256] = \
                res[256 * ch:256 * (ch + 1)].T
    return out


def kernel(x, w_qkv, w_proj, b_proj):
    nc = _get_nc()
    in_maps = _make_in_maps(x, w_qkv, w_proj, b_proj)
    res = run_bass_kernel_spmd(nc, in_maps, core_ids=list(range(8)))
    return _assemble(res.results)


def kernel_traced(x, w_qkv, w_proj, b_proj, **kw):
    """Like kernel() but returns (output, BassKernelResults) with trace."""
    nc = _get_nc()
    in_maps = _make_in_maps(x, w_qkv, w_proj, b_proj)
    res = run_bass_kernel_spmd(nc, in_maps, core_ids=list(range(8)),
                               trace=True, **kw)
    return _assemble(res.results), res

